# revision 3
# baseline (speedup 1.0000x reference)
"""Multi-head attention (B=4, T=2048, C=1024, H=16) on 8 trn2 NeuronCores.

Sharding: core c = 2*b + g handles batch b (of 4) and head-group g (of 2,
8 heads = 512 channels each). Each core computes q/k/v projections for its
512 channels, full TxT attention for its 8 heads, and two partial output
projections out_a = y[heads 0-3] @ Wo-part, out_b = y[heads 4-7] @ Wo-part
(bf16). Host sums the four partials per batch and adds bo.

Mask trick: keys with mask!=0 contribute exactly 0 to softmax (exp(-inf)),
so the host compacts the key axis to the unmasked positions before the K/V
projections (~halves attention work). The compacted key count is padded to a
multiple of 128; padding lanes get a -1e30 bias fused into the exp.

Input DMAs are chunked per 128-row contraction tile and interleaved across
the two queues so the projection units start consuming at ~2us instead of
waiting for whole-tensor transfers.

On-chip layouts (per core):
  xT   [C=1024, T=2048]      x[b].T
  xkT  [C=1024, TKP]         compacted x[b][keep].T
  qT   [512, 2048]           q.T  (head hl = partitions 64*hl..+64 of tile hl//2)
  kT   [512, TKP]            k.T  (same layout)
  vaug [TKP, 8*128]          v in natural layout, 128 cols per head: 64 ones
                             (softmax denominator) + 64 data
  S_T  [Tk tile 128, Tq]     scores transposed: softmax denom = partition sum
  yT   [512, 2048]           normalized attention output transposed
"""

import numpy as np
import ml_dtypes

import concourse.bass as bass
import concourse.mybir as mybir
import concourse.tile as tile
from concourse import bacc
from concourse.bass_utils import run_bass_kernel_spmd

F32 = mybir.dt.float32
BF16 = mybir.dt.bfloat16
NP_BF16 = ml_dtypes.bfloat16

B, T, C = 4, 2048, 1024
H, D = 16, 64
G = 2                 # head groups (cores per batch)
HL = H // G           # heads per core = 8
DL = HL * D           # local channels = 512
SCALE = 1.0 / np.sqrt(D)
NEG = -1e30
N_CORES = 8

_nc_cache: dict = {}


def _build_nc(tkp: int):
    """Build + compile the SPMD Bass program for padded key count tkp."""
    ntk = tkp // 128          # key partition-tiles
    nkc = C // 128            # contraction tiles over C = 8
    nmq = DL // 128           # qT/kT partition-tiles = 4
    assert tkp % 128 == 0

    nc = bacc.Bacc(None, num_swdge_queues=2)

    xT_d = nc.dram_tensor("xT", [C, T], BF16, kind="ExternalInput")
    xkT_d = nc.dram_tensor("xkT", [C, tkp], BF16, kind="ExternalInput")
    wqT_d = nc.dram_tensor("wqT", [C, DL], BF16, kind="ExternalInput")
    wkT_d = nc.dram_tensor("wkT", [C, DL], BF16, kind="ExternalInput")
    wvT_d = nc.dram_tensor("wvT", [C, DL], BF16, kind="ExternalInput")
    woT_d = nc.dram_tensor("woT", [DL, C], BF16, kind="ExternalInput")
    bqp_d = nc.dram_tensor("bqp", [128, nmq], F32, kind="ExternalInput")
    bkp_d = nc.dram_tensor("bkp", [128, nmq], F32, kind="ExternalInput")
    bvp_d = nc.dram_tensor("bvp", [128, DL], F32, kind="ExternalInput")
    mbp_d = nc.dram_tensor("mbp", [128, ntk], F32, kind="ExternalInput")
    outa_d = nc.dram_tensor("out_a", [T, C], BF16, kind="ExternalOutput")
    outb_d = nc.dram_tensor("out_b", [T, C], BF16, kind="ExternalOutput")

    with tile.TileContext(nc) as tc:
        with (
            tc.tile_pool(name="persist", bufs=1) as pp,
            tc.tile_pool(name="work", bufs=4) as wp,
            tc.tile_pool(name="psum", bufs=1, space="PSUM") as psp,
        ):
            # ---- persistent SBUF tensors (wide layout: k-tile k at column k*W) ----
            def persist(shape, dt, tag):
                return pp.tile(shape, dt, tag=tag, name=tag)

            xT_a = persist([128, nkc * T], BF16, "xTa")
            xkT_a = persist([128, nkc * tkp], BF16, "xkTa")
            wqT_a = persist([128, nkc * DL], BF16, "wqTa")
            wkT_a = persist([128, nkc * DL], BF16, "wkTa")
            wvT_a = persist([128, nkc * DL], BF16, "wvTa")
            woT_a = persist([128, nmq * C], BF16, "woTa")
            qp_t = [persist([128, T], BF16, f"qp{h}") for h in range(HL)]
            kT_t = [persist([128, tkp], BF16, f"kT{m}") for m in range(nmq)]
            va_t = [persist([128, HL * 128], BF16, f"va{t}") for t in range(ntk)]
            yT_t = [persist([128, T], BF16, f"yT{m}") for m in range(nmq)]
            bqp_t = persist([128, nmq], F32, "bqp")
            bkp_t = persist([128, nmq], F32, "bkp")
            bvp_t = persist([128, DL], F32, "bvp")
            mbp_t = persist([128, ntk], F32, "mbp")

            # psum slots: "s" x2 (4 banks), "y" x1 (2 banks), "f" x1 (2 banks)
            def psum_tile(shape, tag, name):
                return psp.tile(shape, F32, tag=tag, name=name,
                                bufs=2 if tag == "s" else 1)

            # ---- input DMAs: per-k-tile chunks, alternating the two queues
            # so the projection units chase chunk arrivals.
            engs = [nc.gpsimd, nc.sync]

            nc.sync.dma_start(out=bqp_t[:], in_=bqp_d[:])
            nc.sync.dma_start(out=bkp_t[:], in_=bkp_d[:])
            nc.sync.dma_start(out=bvp_t[:], in_=bvp_d[:])
            nc.sync.dma_start(out=mbp_t[:], in_=mbp_d[:])

            def chunk(dram, sb, W, k):
                engs[k % 2].dma_start(
                    out=sb[:, k * W:(k + 1) * W],
                    in_=dram[k * 128:(k + 1) * 128, :],
                )

            for k in range(nkc):
                chunk(xkT_d[:], xkT_a, tkp, k)
                chunk(wvT_d[:], wvT_a, DL, k)
            for k in range(nkc):
                chunk(wkT_d[:], wkT_a, DL, k)
            for k in range(nkc):
                chunk(wqT_d[:], wqT_a, DL, k)
            for k in range(nkc):
                chunk(xT_d[:], xT_a, T, k)
            for k in range(nmq):
                chunk(woT_d[:], woT_a, C, k)

            for t in range(ntk):
                nc.vector.memset(va_t[t][:], 1.0)  # ones columns of vaug
            for h in range(HL):
                z0 = 0 if h % 2 else 64   # zero the other head's rows
                nc.gpsimd.memset(qp_t[h][z0:z0 + 64, :], 0.0)

            uid = [0]

            # ---- emission units ----
            def v_unit(t, tag):
                uid[0] += 1
                ps = psum_tile([128, DL], tag, f"vps{uid[0]}")
                for k in range(nkc):
                    nc.tensor.matmul(
                        ps[:],
                        lhsT=xkT_a[:, k * tkp + t * 128:k * tkp + (t + 1) * 128],
                        rhs=wvT_a[:, k * DL:(k + 1) * DL],
                        start=(k == 0), stop=(k == nkc - 1),
                    )
                dst = va_t[t][:].rearrange("p (h e) -> p h e", e=128)[:, :, 64:128]
                src = ps[:].rearrange("p (h e) -> p h e", e=D)
                bv3 = bvp_t[:].rearrange("p (h e) -> p h e", e=D)
                nc.vector.tensor_add(dst, src, bv3)

            def q_unit(m, n, tag):  # n: 512-chunk index of T
                uid[0] += 1
                ps = psum_tile([128, 512], tag, f"qps{uid[0]}")
                for k in range(nkc):
                    nc.tensor.matmul(
                        ps[:],
                        lhsT=wqT_a[:, k * DL + m * 128:k * DL + (m + 1) * 128],
                        rhs=xT_a[:, k * T + n * 512:k * T + (n + 1) * 512],
                        start=(k == 0), stop=(k == nkc - 1),
                    )
                for par in range(2):
                    h = 2 * m + par
                    r0 = 64 * par
                    nc.vector.tensor_scalar_add(
                        qp_t[h][r0:r0 + 64, n * 512:(n + 1) * 512],
                        ps[r0:r0 + 64, :], bqp_t[r0:r0 + 64, m:m + 1]
                    )

            def k_unit(m, s0, cn, tag):
                uid[0] += 1
                ps = psum_tile([128, 512], tag, f"kps{uid[0]}")
                for k in range(nkc):
                    nc.tensor.matmul(
                        ps[:, 0:cn],
                        lhsT=wkT_a[:, k * DL + m * 128:k * DL + (m + 1) * 128],
                        rhs=xkT_a[:, k * tkp + s0:k * tkp + s0 + cn],
                        start=(k == 0), stop=(k == nkc - 1),
                    )
                nc.vector.tensor_scalar_add(
                    kT_t[m][:, s0:s0 + cn], ps[:, 0:cn], bkp_t[:, m:m + 1]
                )

            def o_half(mt, half, tag):
                # partial output projection over head-group half (kt pair):
                # out_{a,b}[mt*128:...] = sum_kt yT[kt].T @ woT[kt]
                uid[0] += 1
                dram = outa_d if half == 0 else outb_d
                kts = (0, 1) if half == 0 else (2, 3)
                ps = psum_tile([128, C], tag, f"ops{uid[0]}")
                for h2 in range(2):
                    for i, kt in enumerate(kts):
                        nc.tensor.matmul(
                            ps[:, h2 * 512:(h2 + 1) * 512],
                            lhsT=yT_t[kt][:, mt * 128:(mt + 1) * 128],
                            rhs=woT_a[:, kt * C + h2 * 512:(kt * C + (h2 + 1) * 512)],
                            start=(i == 0), stop=(i == 1),
                        )
                o_sb = wp.tile([128, C], BF16, tag="o", name=f"osb{uid[0]}", bufs=3)
                nc.vector.tensor_copy(o_sb[:], ps[:])
                nc.sync.dma_start(out=dram[mt * 128:(mt + 1) * 128, :], in_=o_sb[:])

            def qk_units(m):
                us = [lambda tag, m=m, s0=s0, cn=min(512, tkp - s0):
                      k_unit(m, s0, cn, tag) for s0 in range(0, tkp, 512)]
                us += [lambda tag, m=m, n=n: q_unit(m, n, tag)
                       for n in range(T // 512)]
                return us

            # ---- startup: v proj + k/q for m=0,1 (chasing the chunked DMAs,
            # in DMA emission order: xkT+wvT, wkT, wqT+xT) ----
            start_units = [lambda tag, t=t: v_unit(t, tag) for t in range(ntk)]
            for m in (0, 1):
                start_units += qk_units(m)
            tags3 = ["s", "s", "f"]
            for i, u in enumerate(start_units):
                u(tags3[i % 3])

            # filler queue consumed inside the attention loop
            fillers = []
            for m in range(2, nmq):
                fillers += qk_units(m)

            EXPF = mybir.ActivationFunctionType.Exp

            def attention(hl, tqc):
                mrow, poff = hl // 2, 64 * (hl % 2)
                q0 = tqc * 1024
                uid[0] += 1
                y_ps = psum_tile([128, 1024], "y", f"yps{uid[0]}")
                pend = []  # software-pipelined PV: lag one t-step behind exp

                def pv(t, p_sb):
                    for h2 in range(2):
                        nc.tensor.matmul(
                            y_ps[:, h2 * 512:(h2 + 1) * 512],
                            lhsT=va_t[t][:, hl * 128:(hl + 1) * 128],
                            rhs=p_sb[:, h2 * 512:(h2 + 1) * 512],
                            start=(t == 0), stop=(t == ntk - 1),
                        )

                for t in range(ntk):
                    uid[0] += 1
                    s_ps = psum_tile([128, 1024], "s", f"sps{uid[0]}")
                    for h2 in range(2):
                        nc.tensor.matmul(
                            s_ps[:, h2 * 512:(h2 + 1) * 512],
                            lhsT=kT_t[mrow][:, t * 128:(t + 1) * 128],
                            rhs=qp_t[hl][:, q0 + h2 * 512:q0 + (h2 + 1) * 512],
                            start=True, stop=True,
                        )
                    p_sb = wp.tile([128, 1024], BF16, tag="p", name=f"p{uid[0]}", bufs=6)
                    nc.scalar.activation(
                        p_sb[:], s_ps[:], EXPF,
                        bias=mbp_t[:, t:t + 1], scale=float(SCALE),
                    )
                    pend.append((t, p_sb))
                    if len(pend) > 1:
                        pv(*pend.pop(0))
                    if fillers and t % 2 == 1:
                        fillers.pop(0)("f")
                while pend:
                    pv(*pend.pop(0))
                # copy numerator+denominator off psum, then normalize from SBUF
                uid[0] += 1
                yu = wp.tile([128, 1024], F32, tag="yu", name=f"yu{uid[0]}", bufs=2)
                nc.vector.tensor_copy(yu[:], y_ps[:])
                rec = wp.tile([1, 1024], F32, tag="rec", name=f"rec{uid[0]}", bufs=2)
                nc.vector.reciprocal_approx_fast(rec[:], yu[0:1, :])
                bc_sb = wp.tile([128, 1024], F32, tag="bc", name=f"bcs{uid[0]}", bufs=2)
                nc.gpsimd.partition_broadcast(bc_sb[:], rec[:])
                nc.vector.tensor_mul(
                    yT_t[mrow][poff:poff + 64, q0:q0 + 1024],
                    yu[64:128, :], bc_sb[64:128, :],
                )

            for tqc in range(2):
                for hl in range(HL):
                    attention(hl, tqc)
                    if hl == 3:
                        # first-half partials for this tqc's queries now ready
                        fillers.extend(
                            lambda tag, mt=mt: o_half(mt, 0, tag)
                            for mt in range(tqc * 8, tqc * 8 + 8)
                        )
                if tqc == 0:
                    # second-half partials for queries 0..1023 ready
                    fillers.extend(
                        lambda tag, mt=mt: o_half(mt, 1, tag)
                        for mt in range(0, 8)
                    )

            # drain remaining fillers, then the tail: out_b for queries
            # 1024..2047 (gated on the last attention call)
            for i, u in enumerate(fillers):
                u("s" if i % 2 else "f")
            for mt in range(8, 16):
                o_half(mt, 1, "s" if mt % 2 else "f")

    nc.compile()
    return nc


def _get_nc(tkp: int):
    if tkp not in _nc_cache:
        _nc_cache[tkp] = _build_nc(tkp)
    return _nc_cache[tkp]


def kernel(x, mask, Wk, bk, Wq, bq, Wv, bv, Wo, bo, _run_kwargs=None):
    x = np.asarray(x, dtype=np.float32)
    mask = np.asarray(mask)
    Wk, bk = np.asarray(Wk, np.float32), np.asarray(bk, np.float32)
    Wq, bq = np.asarray(Wq, np.float32), np.asarray(bq, np.float32)
    Wv, bv = np.asarray(Wv, np.float32), np.asarray(bv, np.float32)
    Wo, bo = np.asarray(Wo, np.float32), np.asarray(bo, np.float32)

    keep = [np.flatnonzero(mask[b] == 0) for b in range(B)]
    max_keep = max(len(kp) for kp in keep)
    tkp = max(128, -(-max_keep // 128) * 128)
    ntk = tkp // 128
    nmq = DL // 128

    nc = _get_nc(tkp)

    in_maps = []
    for b in range(B):
        xT = np.ascontiguousarray(x[b].T).astype(NP_BF16)
        xk = np.zeros((tkp, C), np.float32)
        xk[: len(keep[b])] = x[b][keep[b]]
        xkT = np.ascontiguousarray(xk.T).astype(NP_BF16)
        mb = np.zeros(tkp, np.float32)
        mb[len(keep[b]):] = NEG
        mbp = np.ascontiguousarray(mb.reshape(ntk, 128).T)
        for g in range(G):
            gs, ge = g * DL, (g + 1) * DL
            in_maps.append({
                "xT": xT,
                "xkT": xkT,
                "wqT": np.ascontiguousarray(Wq[gs:ge].T).astype(NP_BF16),
                "wkT": np.ascontiguousarray(Wk[gs:ge].T).astype(NP_BF16),
                "wvT": np.ascontiguousarray(Wv[gs:ge].T).astype(NP_BF16),
                "woT": np.ascontiguousarray(Wo[:, gs:ge].T).astype(NP_BF16),
                "bqp": np.ascontiguousarray(bq[gs:ge].reshape(nmq, 128).T),
                "bkp": np.ascontiguousarray(bk[gs:ge].reshape(nmq, 128).T),
                "bvp": np.ascontiguousarray(
                    np.broadcast_to(bv[gs:ge], (128, DL)).astype(np.float32)),
                "mbp": mbp,
            })

    kw = _run_kwargs or {}
    res = run_bass_kernel_spmd(nc, in_maps, list(range(N_CORES)), **kw)

    out = np.empty((B, T, C), np.float32)
    for b in range(B):
        acc = np.zeros((T, C), np.float32)
        for c in (2 * b, 2 * b + 1):
            acc += np.asarray(res.results[c]["out_a"], np.float32)
            acc += np.asarray(res.results[c]["out_b"], np.float32)
        out[b] = acc + bo
    if kw:
        kernel.last_result = res
    return out


# revision 11
# speedup vs baseline: 1.0361x; 1.0361x over previous
"""Multi-head attention (B=4, T=2048, C=1024, H=16) on 8 trn2 NeuronCores.

Sharding: core c = 2*b + g handles batch b (of 4) and head-group g (of 2,
8 heads = 512 channels each). Each core computes q/k/v projections for its
512 channels, full TxT attention for its 8 heads, and two partial output
projections out_a = y[heads 0-3] @ Wo-part, out_b = y[heads 4-7] @ Wo-part
(bf16). Host sums the four partials per batch and adds bo.

Mask trick: keys with mask!=0 contribute exactly 0 to softmax (exp(-inf)),
so the host compacts the key axis to the unmasked positions before the K/V
projections (~halves attention work). The compacted key count is padded to a
multiple of 128; padding lanes get a -1e30 bias fused into the exp.

Input DMAs are chunked per 128-row contraction tile and interleaved across
the two queues so the projection units start consuming at ~2us instead of
waiting for whole-tensor transfers.

On-chip layouts (per core):
  xT   [C=1024, T=2048]      x[b].T
  xkT  [C=1024, TKP]         compacted x[b][keep].T
  qT   [512, 2048]           q.T  (head hl = partitions 64*hl..+64 of tile hl//2)
  kT   [512, TKP]            k.T  (same layout)
  vaug [TKP, 8*128]          v in natural layout, 128 cols per head: 64 ones
                             (softmax denominator) + 64 data
  S_T  [Tk tile 128, Tq]     scores transposed: softmax denom = partition sum
  yT   [512, 2048]           normalized attention output transposed
"""

import numpy as np
import ml_dtypes

import concourse.bass as bass
import concourse.mybir as mybir
import concourse.tile as tile
from concourse import bacc
from concourse.bass_utils import run_bass_kernel_spmd

F32 = mybir.dt.float32
BF16 = mybir.dt.bfloat16
NP_BF16 = ml_dtypes.bfloat16

B, T, C = 4, 2048, 1024
H, D = 16, 64
G = 2                 # head groups (cores per batch)
HL = H // G           # heads per core = 8
DL = HL * D           # local channels = 512
SCALE = 1.0 / np.sqrt(D)
NEG = -1e30
N_CORES = 8

_nc_cache: dict = {}


def _build_nc(tkp: int):
    """Build + compile the SPMD Bass program for padded key count tkp."""
    ntk = tkp // 128          # key partition-tiles
    nkc = C // 128            # contraction tiles over C = 8
    nmq = DL // 128           # qT/kT partition-tiles = 4
    assert tkp % 128 == 0

    nc = bacc.Bacc(None, num_swdge_queues=2)

    xT_d = nc.dram_tensor("xT", [C, T], BF16, kind="ExternalInput")
    xkT_d = nc.dram_tensor("xkT", [C, tkp], BF16, kind="ExternalInput")
    wqT_d = nc.dram_tensor("wqT", [C, DL], BF16, kind="ExternalInput")
    wkT_d = nc.dram_tensor("wkT", [C, DL], BF16, kind="ExternalInput")
    wvT_d = nc.dram_tensor("wvT", [C, DL], BF16, kind="ExternalInput")
    woT_d = nc.dram_tensor("woT", [DL, C], BF16, kind="ExternalInput")
    bqp_d = nc.dram_tensor("bqp", [128, nmq], F32, kind="ExternalInput")
    bkp_d = nc.dram_tensor("bkp", [128, nmq], F32, kind="ExternalInput")
    bvp_d = nc.dram_tensor("bvp", [128, DL], F32, kind="ExternalInput")
    mbp_d = nc.dram_tensor("mbp", [128, ntk], F32, kind="ExternalInput")
    outa_d = nc.dram_tensor("out_a", [T, C], BF16, kind="ExternalOutput")
    outb_d = nc.dram_tensor("out_b", [T, C], BF16, kind="ExternalOutput")

    with tile.TileContext(nc) as tc:
        with (
            tc.tile_pool(name="persist", bufs=1) as pp,
            tc.tile_pool(name="work", bufs=4) as wp,
            tc.tile_pool(name="psum", bufs=1, space="PSUM") as psp,
        ):
            # ---- persistent SBUF tensors (wide layout: k-tile k at column k*W) ----
            def persist(shape, dt, tag):
                return pp.tile(shape, dt, tag=tag, name=tag)

            xT_a = persist([128, nkc * T], BF16, "xTa")
            xkT_a = persist([128, nkc * tkp], BF16, "xkTa")
            wqT_a = persist([128, nkc * DL], BF16, "wqTa")
            wkT_a = persist([128, nkc * DL], BF16, "wkTa")
            wvT_a = persist([128, nkc * DL], BF16, "wvTa")
            woT_a = persist([128, nmq * C], BF16, "woTa")
            qp_t = [persist([128, T], BF16, f"qp{h}") for h in range(HL)]
            kT_t = [persist([128, tkp], BF16, f"kT{m}") for m in range(nmq)]
            va_t = [persist([128, HL * 128], BF16, f"va{t}") for t in range(ntk)]
            yT_t = [persist([128, T], BF16, f"yT{m}") for m in range(nmq)]
            bqp_t = persist([128, nmq], F32, "bqp")
            bkp_t = persist([128, nmq], F32, "bkp")
            bvp_t = persist([128, DL], F32, "bvp")
            mbp_t = persist([128, ntk], F32, "mbp")

            # psum slots: "s" x3 (6 banks, shared with fillers), "y" x1 (2 banks)
            def psum_tile(shape, tag, name):
                return psp.tile(shape, F32, tag="y" if tag == "y" else "s",
                                name=name, bufs=1 if tag == "y" else 3)

            # ---- input DMAs: per-k-tile chunks, alternating the two queues
            # so the projection units chase chunk arrivals.
            engs = [nc.gpsimd, nc.sync]

            nc.sync.dma_start(out=bqp_t[:], in_=bqp_d[:])
            nc.sync.dma_start(out=bkp_t[:], in_=bkp_d[:])
            nc.sync.dma_start(out=bvp_t[:], in_=bvp_d[:])
            nc.sync.dma_start(out=mbp_t[:], in_=mbp_d[:])

            def chunk(dram, sb, W, c, span=2):
                # one transfer covering k-tiles [c*span, (c+1)*span)
                k0 = c * span
                engs[c % 2].dma_start(
                    out=sb[:, k0 * W:(k0 + span) * W].rearrange(
                        "p (k n) -> p k n", n=W),
                    in_=dram[:].rearrange("(k p) n -> p k n", p=128)[
                        :, k0:k0 + span, :],
                )

            for c in range(nkc // 2):
                chunk(xkT_d, xkT_a, tkp, c)
                chunk(wvT_d, wvT_a, DL, c)
            for c in range(nkc // 2):
                chunk(wkT_d, wkT_a, DL, c)
            for c in range(nkc // 2):
                chunk(wqT_d, wqT_a, DL, c)
            for c in range(nkc // 2):
                chunk(xT_d, xT_a, T, c)
            for c in range(nmq // 2):
                chunk(woT_d, woT_a, C, c)

            for t in range(ntk):
                nc.vector.memset(va_t[t][:], 1.0)  # ones columns of vaug
            for h in range(HL):
                z0 = 0 if h % 2 else 64   # zero the other head's rows
                nc.gpsimd.memset(qp_t[h][z0:z0 + 64, :], 0.0)

            uid = [0]

            # ---- emission units ----
            def v_unit(t, tag):
                uid[0] += 1
                ps = psum_tile([128, DL], tag, f"vps{uid[0]}")
                for k in range(nkc):
                    nc.tensor.matmul(
                        ps[:],
                        lhsT=xkT_a[:, k * tkp + t * 128:k * tkp + (t + 1) * 128],
                        rhs=wvT_a[:, k * DL:(k + 1) * DL],
                        start=(k == 0), stop=(k == nkc - 1),
                    )
                dst = va_t[t][:].rearrange("p (h e) -> p h e", e=128)[:, :, 64:128]
                src = ps[:].rearrange("p (h e) -> p h e", e=D)
                bv3 = bvp_t[:].rearrange("p (h e) -> p h e", e=D)
                nc.vector.tensor_add(dst, src, bv3)

            def q_unit(m, n, tag):  # n: 512-chunk index of T
                uid[0] += 1
                ps = psum_tile([128, 512], tag, f"qps{uid[0]}")
                for k in range(nkc):
                    nc.tensor.matmul(
                        ps[:],
                        lhsT=wqT_a[:, k * DL + m * 128:k * DL + (m + 1) * 128],
                        rhs=xT_a[:, k * T + n * 512:k * T + (n + 1) * 512],
                        start=(k == 0), stop=(k == nkc - 1),
                    )
                for par in range(2):
                    h = 2 * m + par
                    r0 = 64 * par
                    nc.vector.tensor_scalar_add(
                        qp_t[h][r0:r0 + 64, n * 512:(n + 1) * 512],
                        ps[r0:r0 + 64, :], bqp_t[r0:r0 + 64, m:m + 1]
                    )

            def k_unit(m, s0, cn, tag):
                uid[0] += 1
                ps = psum_tile([128, 512], tag, f"kps{uid[0]}")
                for k in range(nkc):
                    nc.tensor.matmul(
                        ps[:, 0:cn],
                        lhsT=wkT_a[:, k * DL + m * 128:k * DL + (m + 1) * 128],
                        rhs=xkT_a[:, k * tkp + s0:k * tkp + s0 + cn],
                        start=(k == 0), stop=(k == nkc - 1),
                    )
                nc.vector.tensor_scalar_add(
                    kT_t[m][:, s0:s0 + cn], ps[:, 0:cn], bkp_t[:, m:m + 1]
                )

            def o_half(mt, half, tag):
                # partial output projection over head-group half (kt pair):
                # out_{a,b}[mt*128:...] = sum_kt yT[kt].T @ woT[kt]
                uid[0] += 1
                dram = outa_d if half == 0 else outb_d
                kts = (0, 1) if half == 0 else (2, 3)
                ps = psum_tile([128, C], tag, f"ops{uid[0]}")
                for h2 in range(2):
                    for i, kt in enumerate(kts):
                        nc.tensor.matmul(
                            ps[:, h2 * 512:(h2 + 1) * 512],
                            lhsT=yT_t[kt][:, mt * 128:(mt + 1) * 128],
                            rhs=woT_a[:, kt * C + h2 * 512:(kt * C + (h2 + 1) * 512)],
                            start=(i == 0), stop=(i == 1),
                        )
                o_sb = wp.tile([128, C], BF16, tag="o", name=f"osb{uid[0]}", bufs=3)
                if mt % 2 and half == 1 and mt >= 8:
                    nc.scalar.copy(o_sb[:], ps[:])
                else:
                    nc.vector.tensor_copy(o_sb[:], ps[:])
                nc.sync.dma_start(out=dram[mt * 128:(mt + 1) * 128, :], in_=o_sb[:])

            def qk_units(m):
                us = [lambda tag, m=m, s0=s0, cn=min(512, tkp - s0):
                      k_unit(m, s0, cn, tag) for s0 in range(0, tkp, 512)]
                us += [lambda tag, m=m, n=n: q_unit(m, n, tag)
                       for n in range(T // 512)]
                return us

            # ---- startup: v proj + k/q for m=0 (chasing the chunked DMAs,
            # in DMA emission order: xkT+wvT, wkT, wqT+xT) ----
            start_units = [lambda tag, t=t: v_unit(t, tag) for t in range(ntk)]
            start_units += qk_units(0)
            for u in start_units:
                u("s")

            # filler queue consumed inside the attention loop
            fillers = []
            for m in range(1, nmq):
                fillers += qk_units(m)

            EXPF = mybir.ActivationFunctionType.Exp

            def attention(hl, tqc, split_norm=False):
                mrow, poff = hl // 2, 64 * (hl % 2)
                q0 = tqc * 1024
                uid[0] += 1
                y_ps = psum_tile([128, 1024], "y", f"yps{uid[0]}")
                pend = []  # software-pipelined PV: lag one t-step behind exp

                def pv(t, p_sb):
                    for h2 in range(2):
                        nc.tensor.matmul(
                            y_ps[:, h2 * 512:(h2 + 1) * 512],
                            lhsT=va_t[t][:, hl * 128:(hl + 1) * 128],
                            rhs=p_sb[:, h2 * 512:(h2 + 1) * 512],
                            start=(t == 0), stop=(t == ntk - 1),
                        )

                for t in range(ntk):
                    uid[0] += 1
                    s_ps = psum_tile([128, 1024], "s", f"sps{uid[0]}")
                    for h2 in range(2):
                        nc.tensor.matmul(
                            s_ps[:, h2 * 512:(h2 + 1) * 512],
                            lhsT=kT_t[mrow][:, t * 128:(t + 1) * 128],
                            rhs=qp_t[hl][:, q0 + h2 * 512:q0 + (h2 + 1) * 512],
                            start=True, stop=True,
                        )
                    p_sb = wp.tile([128, 1024], BF16, tag="p", name=f"p{uid[0]}", bufs=6)
                    nc.scalar.activation(
                        p_sb[:], s_ps[:], EXPF,
                        bias=mbp_t[:, t:t + 1], scale=float(SCALE),
                    )
                    pend.append((t, p_sb))
                    if len(pend) > 1:
                        pv(*pend.pop(0))
                    if fillers and t % 2 == 1:
                        fillers.pop(0)("f")
                while pend:
                    pv(*pend.pop(0))
                # copy numerator+denominator off psum, then normalize from SBUF
                uid[0] += 1
                yu = wp.tile([128, 1024], F32, tag="yu", name=f"yu{uid[0]}", bufs=2)
                rec = wp.tile([1, 1024], F32, tag="rec", name=f"rec{uid[0]}", bufs=2)
                bc_sb = wp.tile([128, 1024], F32, tag="bc", name=f"bcs{uid[0]}", bufs=2)
                # halved chain pipelines DVE/GpSimd to cut the latency until
                # yT is ready (matters for the last call, which gates the tail)
                halves = (0, 512) if split_norm else (0,)
                hw = 1024 // len(halves)
                for c0 in halves:
                    sl = slice(c0, c0 + hw)
                    nc.vector.tensor_copy(yu[:, sl], y_ps[:, sl])
                    nc.vector.reciprocal_approx_fast(rec[0:1, sl], yu[0:1, sl])
                    nc.gpsimd.partition_broadcast(bc_sb[:, sl], rec[0:1, sl])
                    nc.vector.tensor_mul(
                        yT_t[mrow][poff:poff + 64, q0 + c0:q0 + c0 + hw],
                        yu[64:128, sl], bc_sb[64:128, sl],
                    )

            for tqc in range(2):
                for hl in range(HL):
                    attention(hl, tqc, split_norm=(tqc == 1 and hl == HL - 1))
                    if hl == 3:
                        # first-half partials for this tqc's queries now ready
                        fillers.extend(
                            lambda tag, mt=mt: o_half(mt, 0, tag)
                            for mt in range(tqc * 8, tqc * 8 + 8)
                        )
                if tqc == 0:
                    # second-half partials for queries 0..1023 ready
                    fillers.extend(
                        lambda tag, mt=mt: o_half(mt, 1, tag)
                        for mt in range(0, 8)
                    )

            # drain remaining fillers, then the tail: out_b for queries
            # 1024..2047 (gated on the last attention call)
            for i, u in enumerate(fillers):
                u("s" if i % 2 else "f")
            for mt in range(8, 16):
                o_half(mt, 1, "s" if mt % 2 else "f")

    nc.compile()
    return nc


def _get_nc(tkp: int):
    if tkp not in _nc_cache:
        _nc_cache[tkp] = _build_nc(tkp)
    return _nc_cache[tkp]


def kernel(x, mask, Wk, bk, Wq, bq, Wv, bv, Wo, bo, _run_kwargs=None):
    x = np.asarray(x, dtype=np.float32)
    mask = np.asarray(mask)
    Wk, bk = np.asarray(Wk, np.float32), np.asarray(bk, np.float32)
    Wq, bq = np.asarray(Wq, np.float32), np.asarray(bq, np.float32)
    Wv, bv = np.asarray(Wv, np.float32), np.asarray(bv, np.float32)
    Wo, bo = np.asarray(Wo, np.float32), np.asarray(bo, np.float32)

    keep = [np.flatnonzero(mask[b] == 0) for b in range(B)]
    max_keep = max(len(kp) for kp in keep)
    tkp = max(128, -(-max_keep // 128) * 128)
    ntk = tkp // 128
    nmq = DL // 128

    nc = _get_nc(tkp)

    in_maps = []
    for b in range(B):
        xT = np.ascontiguousarray(x[b].T).astype(NP_BF16)
        xk = np.zeros((tkp, C), np.float32)
        xk[: len(keep[b])] = x[b][keep[b]]
        xkT = np.ascontiguousarray(xk.T).astype(NP_BF16)
        mb = np.zeros(tkp, np.float32)
        mb[len(keep[b]):] = NEG
        mbp = np.ascontiguousarray(mb.reshape(ntk, 128).T)
        for g in range(G):
            gs, ge = g * DL, (g + 1) * DL
            in_maps.append({
                "xT": xT,
                "xkT": xkT,
                "wqT": np.ascontiguousarray(Wq[gs:ge].T).astype(NP_BF16),
                "wkT": np.ascontiguousarray(Wk[gs:ge].T).astype(NP_BF16),
                "wvT": np.ascontiguousarray(Wv[gs:ge].T).astype(NP_BF16),
                "woT": np.ascontiguousarray(Wo[:, gs:ge].T).astype(NP_BF16),
                "bqp": np.ascontiguousarray(bq[gs:ge].reshape(nmq, 128).T),
                "bkp": np.ascontiguousarray(bk[gs:ge].reshape(nmq, 128).T),
                "bvp": np.ascontiguousarray(
                    np.broadcast_to(bv[gs:ge], (128, DL)).astype(np.float32)),
                "mbp": mbp,
            })

    kw = _run_kwargs or {}
    res = run_bass_kernel_spmd(nc, in_maps, list(range(N_CORES)), **kw)

    out = np.empty((B, T, C), np.float32)
    for b in range(B):
        acc = np.zeros((T, C), np.float32)
        for c in (2 * b, 2 * b + 1):
            acc += np.asarray(res.results[c]["out_a"], np.float32)
            acc += np.asarray(res.results[c]["out_b"], np.float32)
        out[b] = acc + bo
    if kw:
        kernel.last_result = res
    return out


# revision 23
# speedup vs baseline: 1.0523x; 1.0156x over previous
"""Multi-head attention (B=4, T=2048, C=1024, H=16) on 8 trn2 NeuronCores.

Sharding: core c = 2*b + g handles batch b (of 4) and head-group g (of 2,
8 heads = 512 channels each). Each core computes q/k/v projections for its
512 channels, full TxT attention for its 8 heads, and two partial output
projections out_a = y[heads 0-3] @ Wo-part, out_b = y[heads 4-7] @ Wo-part
(bf16). Host sums the four partials per batch and adds bo.

Mask trick: keys with mask!=0 contribute exactly 0 to softmax (exp(-inf)),
so the host compacts the key axis to the unmasked positions before the K/V
projections (~halves attention work). The compacted key count is padded to a
multiple of 128; padding lanes get a -1e30 bias fused into the exp.

Input DMAs are chunked per 128-row contraction tile and interleaved across
the two queues so the projection units start consuming at ~2us instead of
waiting for whole-tensor transfers.

On-chip layouts (per core):
  xT   [C=1024, T=2048]      x[b].T
  xkT  [C=1024, TKP]         compacted x[b][keep].T
  qT   [512, 2048]           q.T  (head hl = partitions 64*hl..+64 of tile hl//2)
  kT   [512, TKP]            k.T  (same layout)
  vaug [TKP, 8*128]          v in natural layout, 128 cols per head: 64 ones
                             (softmax denominator) + 64 data
  S_T  [Tk tile 128, Tq]     scores transposed: softmax denom = partition sum
  yT   [512, 2048]           normalized attention output transposed
"""

import numpy as np
import ml_dtypes

import concourse.bass as bass
import concourse.mybir as mybir
import concourse.tile as tile
from concourse import bacc
from concourse.bass_utils import run_bass_kernel_spmd

F32 = mybir.dt.float32
BF16 = mybir.dt.bfloat16
NP_BF16 = ml_dtypes.bfloat16

B, T, C = 4, 2048, 1024
H, D = 16, 64
G = 2                 # head groups (cores per batch)
HL = H // G           # heads per core = 8
DL = HL * D           # local channels = 512
SCALE = 1.0 / np.sqrt(D)
NEG = -1e30
N_CORES = 8

_nc_cache: dict = {}


def _build_nc(tkp: int):
    """Build + compile the SPMD Bass program for padded key count tkp."""
    ntk = tkp // 128          # key partition-tiles
    nkc = C // 128            # contraction tiles over C = 8
    nmq = DL // 128           # qT/kT partition-tiles = 4
    assert tkp % 128 == 0

    nc = bacc.Bacc(None, num_swdge_queues=2)

    # inputs are pre-arranged host-side to [128, ktiles*W] (partition-major,
    # k-tile k at columns k*W..) so chunk DMAs are contiguous 2D copies
    xT_d = nc.dram_tensor("xT", [128, (C // 128) * T], BF16, kind="ExternalInput")
    xkT_d = nc.dram_tensor("xkT", [128, (C // 128) * tkp], BF16, kind="ExternalInput")
    wqT_d = nc.dram_tensor("wqT", [128, (C // 128) * DL], BF16, kind="ExternalInput")
    wkT_d = nc.dram_tensor("wkT", [128, (C // 128) * DL], BF16, kind="ExternalInput")
    wvT_d = nc.dram_tensor("wvT", [128, (C // 128) * DL], BF16, kind="ExternalInput")
    woT_d = nc.dram_tensor("woT", [128, (DL // 128) * C], BF16, kind="ExternalInput")
    bqp_d = nc.dram_tensor("bqp", [128, nmq], F32, kind="ExternalInput")
    bkp_d = nc.dram_tensor("bkp", [128, nmq], F32, kind="ExternalInput")
    bvp_d = nc.dram_tensor("bvp", [128, DL], F32, kind="ExternalInput")
    mbp_d = nc.dram_tensor("mbp", [128, ntk], F32, kind="ExternalInput")
    outa_d = nc.dram_tensor("out_a", [T, C], BF16, kind="ExternalOutput")
    outb_d = nc.dram_tensor("out_b", [T, C], BF16, kind="ExternalOutput")

    with tile.TileContext(nc) as tc:
        with (
            tc.tile_pool(name="persist", bufs=1) as pp,
            tc.tile_pool(name="work", bufs=4) as wp,
            tc.tile_pool(name="psum", bufs=1, space="PSUM") as psp,
        ):
            # ---- persistent SBUF tensors (wide layout: k-tile k at column k*W) ----
            def persist(shape, dt, tag):
                return pp.tile(shape, dt, tag=tag, name=tag)

            xT_a = persist([128, nkc * T], BF16, "xTa")
            xkT_a = persist([128, nkc * tkp], BF16, "xkTa")
            wqT_a = persist([128, nkc * DL], BF16, "wqTa")
            wkT_a = persist([128, nkc * DL], BF16, "wkTa")
            wvT_a = persist([128, nkc * DL], BF16, "wvTa")
            woT_a = persist([128, nmq * C], BF16, "woTa")
            qp_t = [persist([128, T], BF16, f"qp{h}") for h in range(HL)]
            kT_t = [persist([128, tkp], BF16, f"kT{m}") for m in range(nmq)]
            va_t = [persist([128, HL * 128], BF16, f"va{t}") for t in range(ntk)]
            yT_t = [persist([128, T], BF16, f"yT{m}") for m in range(nmq)]
            bqp_t = persist([128, nmq], F32, "bqp")
            bkp_t = persist([128, nmq], F32, "bkp")
            bvp_t = persist([128, DL], F32, "bvp")
            mbp_t = persist([128, ntk], F32, "mbp")

            # psum slots: "s" x3 (6 banks, shared with fillers), "y" x1 (2 banks)
            # all tiles are uniformly [128, 1024] (2 banks) so the shared
            # rotation never packs different footprints into live slots
            def psum_tile(shape, tag, name):
                assert shape == [128, 1024]
                return psp.tile(shape, F32, tag="y" if tag == "y" else "s",
                                name=name, bufs=1 if tag == "y" else 3)

            # ---- input DMAs: contiguous 2-k-tile chunks round-robined over
            # the three DMA-capable engine queues (sync/scalar HWDGE + gpsimd
            # SWDGE) so descriptor generation parallelizes and the projection
            # units chase chunk arrivals.
            engs = [nc.sync, nc.gpsimd, nc.scalar]
            eng_i = [0]

            nc.sync.dma_start(out=bqp_t[:], in_=bqp_d[:])
            nc.gpsimd.dma_start(out=bkp_t[:], in_=bkp_d[:])
            nc.scalar.dma_start(out=bvp_t[:], in_=bvp_d[:])
            nc.sync.dma_start(out=mbp_t[:], in_=mbp_d[:])

            def chunk(dram, sb, W, c, span=2):
                # one transfer covering k-tiles [c*span, (c+1)*span)
                c0, c1 = c * span * W, (c + 1) * span * W
                engs[eng_i[0] % 3].dma_start(out=sb[:, c0:c1], in_=dram[:, c0:c1])
                eng_i[0] += 1

            for c in range(nkc // 2):
                chunk(xkT_d, xkT_a, tkp, c)
                chunk(wvT_d, wvT_a, DL, c)
            for c in range(nkc // 2):
                chunk(wkT_d, wkT_a, DL, c)
            for c in range(nkc // 2):
                chunk(wqT_d, wqT_a, DL, c)
            for c in range(nkc // 2):
                chunk(xT_d, xT_a, T, c)
            for c in range(nmq // 2):
                chunk(woT_d, woT_a, C, c)

            for t in range(ntk):
                nc.vector.memset(va_t[t][:], 1.0)  # ones columns of vaug
            for h in range(HL):
                z0 = 0 if h % 2 else 64   # zero the other head's rows
                nc.gpsimd.memset(qp_t[h][z0:z0 + 64, :], 0.0)

            uid = [0]

            # ---- emission units ----
            def v_unit(t, tag):
                uid[0] += 1
                ps = psum_tile([128, 1024], tag, f"vps{uid[0]}")
                for k in range(nkc):
                    nc.tensor.matmul(
                        ps[:, 0:DL],
                        lhsT=xkT_a[:, k * tkp + t * 128:k * tkp + (t + 1) * 128],
                        rhs=wvT_a[:, k * DL:(k + 1) * DL],
                        start=(k == 0), stop=(k == nkc - 1),
                    )
                dst = va_t[t][:].rearrange("p (h e) -> p h e", e=128)[:, :, 64:128]
                src = ps[:, 0:DL].rearrange("p (h e) -> p h e", e=D)
                bv3 = bvp_t[:].rearrange("p (h e) -> p h e", e=D)
                nc.vector.tensor_add(dst, src, bv3)

            def q_unit(m, s0, cn, tag):  # query columns [s0, s0+cn)
                uid[0] += 1
                ps = psum_tile([128, 1024], tag, f"qps{uid[0]}")
                for k in range(nkc):
                    nc.tensor.matmul(
                        ps[:, 0:cn],
                        lhsT=wqT_a[:, k * DL + m * 128:k * DL + (m + 1) * 128],
                        rhs=xT_a[:, k * T + s0:k * T + s0 + cn],
                        start=(k == 0), stop=(k == nkc - 1),
                    )
                for par in range(2):
                    h = 2 * m + par
                    r0 = 64 * par
                    nc.vector.tensor_scalar_add(
                        qp_t[h][r0:r0 + 64, s0:s0 + cn],
                        ps[r0:r0 + 64, 0:cn], bqp_t[r0:r0 + 64, m:m + 1]
                    )

            def k_unit(m, s0, cn, tag):
                uid[0] += 1
                ps = psum_tile([128, 1024], tag, f"kps{uid[0]}")
                for k in range(nkc):
                    nc.tensor.matmul(
                        ps[:, 0:cn],
                        lhsT=wkT_a[:, k * DL + m * 128:k * DL + (m + 1) * 128],
                        rhs=xkT_a[:, k * tkp + s0:k * tkp + s0 + cn],
                        start=(k == 0), stop=(k == nkc - 1),
                    )
                nc.vector.tensor_scalar_add(
                    kT_t[m][:, s0:s0 + cn], ps[:, 0:cn], bkp_t[:, m:m + 1]
                )

            def o_half(mt, half, tag):
                # partial output projection over head-group half (kt pair):
                # out_{a,b}[mt*128:...] = sum_kt yT[kt].T @ woT[kt]
                uid[0] += 1
                dram = outa_d if half == 0 else outb_d
                kts = (0, 1) if half == 0 else (2, 3)
                ps = psum_tile([128, 1024], tag, f"ops{uid[0]}")
                for h2 in range(2):
                    for i, kt in enumerate(kts):
                        nc.tensor.matmul(
                            ps[:, h2 * 512:(h2 + 1) * 512],
                            lhsT=yT_t[kt][:, mt * 128:(mt + 1) * 128],
                            rhs=woT_a[:, kt * C + h2 * 512:(kt * C + (h2 + 1) * 512)],
                            start=(i == 0), stop=(i == 1),
                        )
                o_sb = wp.tile([128, C], BF16, tag="o", name=f"osb{uid[0]}", bufs=3)
                if mt % 2 and half == 1 and mt >= 8:
                    nc.scalar.copy(o_sb[:], ps[:])
                else:
                    nc.vector.tensor_copy(o_sb[:], ps[:])
                nc.sync.dma_start(out=dram[mt * 128:(mt + 1) * 128, :], in_=o_sb[:])

            def qk_units(m, step=512):
                us = [lambda tag, m=m, s0=s0, cn=min(step, tkp - s0):
                      k_unit(m, s0, cn, tag) for s0 in range(0, tkp, step)]
                us += [lambda tag, m=m, s0=s0, cn=step: q_unit(m, s0, cn, tag)
                       for s0 in range(0, T, step)]
                return us

            # ---- startup: v proj + k/q for m=0 (chasing the chunked DMAs,
            # in DMA emission order: xkT+wvT, wkT, wqT+xT) ----
            start_units = [lambda tag, t=t: v_unit(t, tag) for t in range(ntk)]
            start_units += qk_units(0)
            for u in start_units:
                u("s")

            # filler queue consumed inside the attention loop. NOTE: keep the
            # unit width at 512 — the attention scores matmul reads qp in
            # 512-wide spans, and writes narrower than the reads that consume
            # them have produced dependency races (wrong/NaN heads 4-7).
            fillers = []
            for m in range(1, nmq):
                fillers += qk_units(m, step=512)

            EXPF = mybir.ActivationFunctionType.Exp

            def attention(hl, tqc, split_norm=False):
                mrow, poff = hl // 2, 64 * (hl % 2)
                q0 = tqc * 1024
                uid[0] += 1
                y_ps = psum_tile([128, 1024], "y", f"yps{uid[0]}")
                pend = []  # software-pipelined PV: lag one t-step behind exp

                def pv(t, p_sb):
                    for h2 in range(2):
                        nc.tensor.matmul(
                            y_ps[:, h2 * 512:(h2 + 1) * 512],
                            lhsT=va_t[t][:, hl * 128:(hl + 1) * 128],
                            rhs=p_sb[:, h2 * 512:(h2 + 1) * 512],
                            start=(t == 0), stop=(t == ntk - 1),
                        )

                for t in range(ntk):
                    uid[0] += 1
                    s_ps = psum_tile([128, 1024], "s", f"sps{uid[0]}")
                    for h2 in range(2):
                        nc.tensor.matmul(
                            s_ps[:, h2 * 512:(h2 + 1) * 512],
                            lhsT=kT_t[mrow][:, t * 128:(t + 1) * 128],
                            rhs=qp_t[hl][:, q0 + h2 * 512:q0 + (h2 + 1) * 512],
                            start=True, stop=True,
                        )
                    p_sb = wp.tile([128, 1024], BF16, tag="p", name=f"p{uid[0]}", bufs=6)
                    nc.scalar.activation(
                        p_sb[:], s_ps[:], EXPF,
                        bias=mbp_t[:, t:t + 1], scale=float(SCALE),
                    )
                    pend.append((t, p_sb))
                    if len(pend) > 1:
                        pv(*pend.pop(0))
                    if len(fillers) > 5 and t % 2 == 1:
                        fillers.pop(0)("f")
                while pend:
                    pv(*pend.pop(0))
                # copy numerator+denominator off psum, then normalize from SBUF
                uid[0] += 1
                yu = wp.tile([128, 1024], F32, tag="yu", name=f"yu{uid[0]}", bufs=2)
                rec = wp.tile([1, 1024], F32, tag="rec", name=f"rec{uid[0]}", bufs=2)
                bc_sb = wp.tile([128, 1024], F32, tag="bc", name=f"bcs{uid[0]}", bufs=2)
                # halved chain pipelines DVE/GpSimd to cut the latency until
                # yT is ready (matters for the last call, which gates the tail)
                halves = (0, 512) if split_norm else (0,)
                hw = 1024 // len(halves)
                for c0 in halves:
                    sl = slice(c0, c0 + hw)
                    nc.vector.tensor_copy(yu[:, sl], y_ps[:, sl])
                    nc.vector.reciprocal_approx_fast(rec[0:1, sl], yu[0:1, sl])
                    nc.gpsimd.partition_broadcast(bc_sb[:, sl], rec[0:1, sl])
                    nc.vector.tensor_mul(
                        yT_t[mrow][poff:poff + 64, q0 + c0:q0 + c0 + hw],
                        yu[64:128, sl], bc_sb[64:128, sl],
                    )

            for tqc in range(2):
                for hl in range(HL):
                    attention(hl, tqc, split_norm=(tqc == 1 and hl == HL - 1))
                    if hl == 3:
                        # first-half partials for this tqc's queries now ready
                        fillers.extend(
                            lambda tag, mt=mt: o_half(mt, 0, tag)
                            for mt in range(tqc * 8, tqc * 8 + 8)
                        )
                if tqc == 0:
                    # second-half partials for queries 0..1023 ready
                    fillers.extend(
                        lambda tag, mt=mt: o_half(mt, 1, tag)
                        for mt in range(0, 8)
                    )

            # drain remaining fillers, then the tail: out_b for queries
            # 1024..2047 (gated on the last attention call)
            for i, u in enumerate(fillers):
                u("s" if i % 2 else "f")
            for mt in range(8, 16):
                o_half(mt, 1, "s" if mt % 2 else "f")

    nc.compile()
    return nc


def _get_nc(tkp: int):
    if tkp not in _nc_cache:
        _nc_cache[tkp] = _build_nc(tkp)
    return _nc_cache[tkp]


def kernel(x, mask, Wk, bk, Wq, bq, Wv, bv, Wo, bo, _run_kwargs=None):
    x = np.asarray(x, dtype=np.float32)
    mask = np.asarray(mask)
    Wk, bk = np.asarray(Wk, np.float32), np.asarray(bk, np.float32)
    Wq, bq = np.asarray(Wq, np.float32), np.asarray(bq, np.float32)
    Wv, bv = np.asarray(Wv, np.float32), np.asarray(bv, np.float32)
    Wo, bo = np.asarray(Wo, np.float32), np.asarray(bo, np.float32)

    keep = [np.flatnonzero(mask[b] == 0) for b in range(B)]
    max_keep = max(len(kp) for kp in keep)
    tkp = max(128, -(-max_keep // 128) * 128)
    ntk = tkp // 128
    nmq = DL // 128

    nc = _get_nc(tkp)

    def pmajor(a):
        # [K, W] -> [128, (K//128)*W]: k-tile k at columns k*W (partition-major)
        K, W = a.shape
        return np.ascontiguousarray(
            a.reshape(K // 128, 128, W).transpose(1, 0, 2).reshape(128, -1)
        ).astype(NP_BF16)

    in_maps = []
    for b in range(B):
        xT = pmajor(x[b].T)
        xk = np.zeros((tkp, C), np.float32)
        xk[: len(keep[b])] = x[b][keep[b]]
        xkT = pmajor(xk.T)
        mb = np.zeros(tkp, np.float32)
        mb[len(keep[b]):] = NEG
        mbp = np.ascontiguousarray(mb.reshape(ntk, 128).T)
        for g in range(G):
            gs, ge = g * DL, (g + 1) * DL
            in_maps.append({
                "xT": xT,
                "xkT": xkT,
                "wqT": pmajor(Wq[gs:ge].T),
                "wkT": pmajor(Wk[gs:ge].T),
                "wvT": pmajor(Wv[gs:ge].T),
                "woT": pmajor(Wo[:, gs:ge].T),
                "bqp": np.ascontiguousarray(bq[gs:ge].reshape(nmq, 128).T),
                "bkp": np.ascontiguousarray(bk[gs:ge].reshape(nmq, 128).T),
                "bvp": np.ascontiguousarray(
                    np.broadcast_to(bv[gs:ge], (128, DL)).astype(np.float32)),
                "mbp": mbp,
            })

    kw = _run_kwargs or {}
    res = run_bass_kernel_spmd(nc, in_maps, list(range(N_CORES)), **kw)

    out = np.empty((B, T, C), np.float32)
    for b in range(B):
        acc = np.zeros((T, C), np.float32)
        for c in (2 * b, 2 * b + 1):
            acc += np.asarray(res.results[c]["out_a"], np.float32)
            acc += np.asarray(res.results[c]["out_b"], np.float32)
        out[b] = acc + bo
    if kw:
        kernel.last_result = res
    return out
